# revision 26
# baseline (speedup 1.0000x reference)
"""Trainium2 Bass kernel for nn_AdaptiveSampler.

Per batch element b of BT=64:
  1. seed  = bilinear_sample(features[b], keypoints[b])          # [C, J]
  2. h     = relu(w1 @ seed + b1); off = w2 @ h + b2             # [2N, J] px
  3. samp  = bilinear_sample(features[b], keypoints + off)       # [C, J*N]
  4. out[b] = samp rearranged to [J, N*C]

Data-parallel over BT across 8 NeuronCores (8 batches/core, 4 pairs).
Features are host-transposed to channel-quad-last layout: SBUF pair tile
[128 part = (b_lo, c//4), free = (y, x, c%4)], loaded once (memory roofline
~90us/core).  Bilinear gathers use the gpsimd `ap_gather` extended
instruction with d=4 (one int16 index per (sample, neighbor) fetches 4
channels), with per-16-partition index groups carrying each b_lo's indices.
Second-pass indices/weights are computed on-chip from the MLP output
(floor/clip via the 2^23 trick on DVE), wrapped into the [32, n/16] storage
layout by one SBUF DMA and replicated across partitions with one PE matmul.
The 4-neighbor combine is one DVE multiply (step-0 channel broadcast of the
weight tile in PSUM) + one strided reduce.  Output [J*N, C] comes from PE
transposes.
"""

import os
import sys

import numpy as np

sys.path.insert(0, "/opt/trn_rl_repo")

BT, C, H, W = 64, 256, 64, 64
J, NPTS = 17, 4
NCORES = 8
BPC = BT // NCORES          # 8 batches per core
NPAIR = BPC // 2
O2 = 2 * NPTS
PT1 = 20                    # padded pass-1 points per b (17 -> 20)
N1 = PT1 * 4                # pass-1 gather slots per group = 80 (F=5)
PTN = J * NPTS              # pass-2 samples per b = 68
N2 = PTN * 4                # pass-2 gather slots per group = 272 (F=17)
FREE = 4096                 # d=4 units per partition (y, x)
MAGIC = 8388608.0

_CACHE = {}
LAST_RESULTS = None


def _build():
    import concourse.bass as bass
    import concourse.tile as tile
    from concourse import bacc, mybir
    from concourse.ap import AP

    dt = mybir.dt
    f32 = dt.float32
    i16 = dt.int16
    Alu = mybir.AluOpType
    Act = mybir.ActivationFunctionType

    nc = bacc.Bacc("TRN2", target_bir_lowering=False, debug=False,
                   num_devices=NCORES)

    feats = nc.dram_tensor("features", [BPC, 64, 4 * H * W], f32,
                           kind="ExternalInput").ap()
    basep = nc.dram_tensor("base_pix", [1, NPAIR * 2 * 2 * PTN], f32,
                           kind="ExternalInput").ap()
    w1qd = nc.dram_tensor("w1q", [128, 512], f32, kind="ExternalInput").ap()
    w2Td = nc.dram_tensor("w2T", [128, O2], f32, kind="ExternalInput").ap()
    b1d = nc.dram_tensor("b1", [128, 1], f32, kind="ExternalInput").ap()
    b2d = nc.dram_tensor("b2", [128, O2], f32, kind="ExternalInput").ap()
    replbd = nc.dram_tensor("replb", [32, 128], f32,
                            kind="ExternalInput").ap()
    onesbd = nc.dram_tensor("onesb", [2, 128], f32, kind="ExternalInput").ap()
    ident2d = nc.dram_tensor("ident2", [128, 64], f32,
                             kind="ExternalInput").ap()
    out = nc.dram_tensor("out", [BPC, J, NPTS * C], f32,
                         kind="ExternalOutput").ap()

    out_v = out.rearrange("b j (n c) -> b (j n) c", c=C)

    from contextlib import ExitStack

    with tile.TileContext(nc) as tc, ExitStack() as ctx:
        const = ctx.enter_context(tc.tile_pool(name="const", bufs=1))
        featp = ctx.enter_context(tc.tile_pool(name="featp", bufs=2))
        gath = ctx.enter_context(tc.tile_pool(name="gath", bufs=2))
        rowp = ctx.enter_context(tc.tile_pool(name="rowp", bufs=1))
        wbp = ctx.enter_context(tc.tile_pool(name="wbp", bufs=2))
        outp = ctx.enter_context(tc.tile_pool(name="outp", bufs=2))
        psum = ctx.enter_context(tc.tile_pool(name="psum", bufs=2,
                                              space="PSUM"))
        psum1 = ctx.enter_context(tc.tile_pool(name="psum1", bufs=1,
                                               space="PSUM"))

        # ---- constants ----
        w1q_t = const.tile([128, 512], f32, tag="w1q")
        nc.sync.dma_start(w1q_t[:], w1qd)
        w2_t = const.tile([128, O2], f32, tag="w2")
        nc.sync.dma_start(w2_t[:], w2Td)
        b1_t = const.tile([128, 1], f32, tag="b1")
        nc.sync.dma_start(b1_t[:], b1d)
        b2_t = const.tile([128, O2], f32, tag="b2")
        nc.sync.dma_start(b2_t[:], b2d)
        replb_t = const.tile([32, 128], f32, tag="replb")
        nc.sync.dma_start(replb_t[:], replbd)
        onesb_t = const.tile([2, 128], f32, tag="onesb")
        nc.sync.dma_start(onesb_t[:], onesbd)
        ident2_t = const.tile([128, 64], f32, tag="ident2")
        nc.sync.dma_start(ident2_t[:], ident2d)
        base_t = const.tile([1, NPAIR * 2 * 2 * PTN], f32, tag="base")
        nc.sync.dma_start(base_t[:], basep)

        def chain(pool, x, n, tagpfx):
            """floor/clip/validity-weights on a [1, n] row; returns
            (c0, c1, w0, w1) tiles [1, n]."""
            s1 = pool.tile([1, n], f32, tag=tagpfx + "s1")
            s2 = pool.tile([1, n], f32, tag=tagpfx + "s2")
            s3 = pool.tile([1, n], f32, tag=tagpfx + "s3")
            c0 = pool.tile([1, n], f32, tag=tagpfx + "c0")
            c1 = pool.tile([1, n], f32, tag=tagpfx + "c1")
            w0 = pool.tile([1, n], f32, tag=tagpfx + "w0")
            w1t = pool.tile([1, n], f32, tag=tagpfx + "w1")
            hi = float(W - 1)
            nc.vector.tensor_scalar(s1[:], x, MAGIC, None, Alu.add)
            nc.vector.tensor_scalar(s1[:], s1[:], MAGIC, None, Alu.subtract)
            nc.vector.tensor_tensor(s2[:], x, s1[:], Alu.is_lt)
            nc.vector.tensor_sub(s1[:], s1[:], s2[:])              # floor
            nc.vector.tensor_scalar(s3[:], s1[:], 1.0, None, Alu.add)
            nc.vector.tensor_scalar(c0[:], s1[:], 0.0, hi, Alu.max, Alu.min)
            nc.vector.tensor_scalar(c1[:], s3[:], 0.0, hi, Alu.max, Alu.min)
            nc.vector.tensor_tensor(s2[:], s1[:], c0[:], Alu.is_equal)
            nc.vector.tensor_sub(w0[:], s3[:], x)
            nc.vector.tensor_mul(w0[:], w0[:], s2[:])
            nc.vector.tensor_tensor(s2[:], s3[:], c1[:], Alu.is_equal)
            nc.vector.tensor_sub(w1t[:], x, s1[:])
            nc.vector.tensor_mul(w1t[:], w1t[:], s2[:])
            return c0, c1, w0, w1t

        def assemble(pool, npt, cx, cy, wx, wy, xoff, tagpfx):
            """Build e-order idx row [1, 2*npt*4] (b_lo-major halves) and
            slot-order weight row [1, 2*npt*4].

            Per group slot i = pt*4 + q; storage row p = i%16 holds
            e = F*p + i//16 with F = npt/4.  cx/cy/wx/wy are (lo, hi)
            [1, *] rows; samples for (b_lo, axis) start at col
            xoff(b_lo, axis).
            """
            F = npt // 4
            ni = 2 * npt * 4
            idxe = pool.tile([1, ni], f32, tag=tagpfx + "idx")
            wrow = pool.tile([1, ni], f32, tag=tagpfx + "wrow")
            it, io = idxe[:].tensor, idxe[:].offset
            wt, wo = wrow[:].tensor, wrow[:].offset
            for b_lo in range(2):
                for q in range(4):
                    qy, qx = q // 2, q % 2
                    # e-grid: [pm = pt%4 (4), pd = pt//4 (F)]
                    ysrc = AP(cy[qy].tensor, cy[qy].offset + xoff(b_lo, 1),
                              [list(cy[qy].ap[0]), [1, 4], [4, F]])
                    xsrc = AP(cx[qx].tensor, cx[qx].offset + xoff(b_lo, 0),
                              [list(cx[qx].ap[0]), [1, 4], [4, F]])
                    idst = AP(it, io + b_lo * npt * 4 + F * q,
                              [[ni, 1], [4 * F, 4], [1, F]])
                    nc.vector.scalar_tensor_tensor(idst, ysrc, float(W),
                                                   xsrc, Alu.mult, Alu.add)
                    wysrc = AP(wy[qy].tensor, wy[qy].offset + xoff(b_lo, 1),
                               [list(wy[qy].ap[0]), [1, npt]])
                    wxsrc = AP(wx[qx].tensor, wx[qx].offset + xoff(b_lo, 0),
                               [list(wx[qx].ap[0]), [1, npt]])
                    wdst = AP(wt, wo + b_lo * npt * 4 + q,
                              [[ni, 1], [4, npt]])
                    nc.vector.tensor_mul(wdst, wysrc, wxsrc)
            return idxe, wrow

        def wrap_idx(pool, idx_row, ni, tagpfx):
            """e-order f32 idx row [1, ni] -> int16 idx tile [128, ni/32]."""
            nf = ni // 32
            wrapt = pool.tile([32, nf], f32, tag=tagpfx + "wrap")
            nc.sync.dma_start(wrapt[:], idx_row[:])
            rep_ps = psum1.tile([128, nf], f32, tag="replps")
            nc.tensor.matmul(rep_ps[:], replb_t[:], wrapt[:], start=True,
                             stop=True)
            idxt = pool.tile([128, nf], i16, tag=tagpfx + "idxi")
            nc.vector.tensor_copy(idxt[:], rep_ps[:])
            return idxt

        def wrap_w(pool, w_row, ni, tagpfx):
            """slot-order w row [1, ni] -> [2, ni/2] tile (b_lo rows)."""
            wpair = pool.tile([2, ni // 2], f32, tag=tagpfx + "wpair")
            nc.sync.dma_start(wpair[:], w_row[:])
            return wpair

        def repl_w(wpair, nf):
            wps = psum1.tile([128, nf], f32, tag="wps")
            nc.tensor.matmul(wps[:], onesb_t[:], wpair[:], start=True,
                             stop=True)
            return wps

        def combine(g, wps, npt, nslots):
            """g [128, nslots*4] (slot, cm) *= w[slot]; reduce over q ->
            [128, npt*4] cols (pt, cm)."""
            gv = g[:].rearrange("p (s c) -> p s c", c=4)
            wb = AP(wps[:].tensor, wps[:].offset,
                    [list(wps[:].ap[0]), [1, nslots], [0, 4]])
            nc.vector.tensor_mul(gv, gv, wb)
            red = gath.tile([128, npt * 4], f32, tag=f"red{nslots}")
            rin = AP(g[:].tensor, g[:].offset,
                     [list(g[:].ap[0]), [16, npt], [1, 4], [4, 4]])
            nc.vector.reduce_sum(red[:].rearrange("p (s c) -> p s c", c=4),
                                 rin, axis=mybir.AxisListType.X)
            return red

        # ---- pass-1 prep (coords only) ----
        base_v = base_t[:].rearrange("a (k x s n) -> a k x s n", k=NPAIR,
                                     x=2, n=4)
        idx1 = []
        w1pair = []
        for k in range(NPAIR):
            xy1 = rowp.tile([1, 4 * PT1], f32, tag="p1xy")
            nc.vector.memset(xy1[:], 0.0)
            for axis in range(2):
                for b_lo in range(2):
                    dst = xy1[:, axis * 2 * PT1 + b_lo * PT1:
                              axis * 2 * PT1 + b_lo * PT1 + J]
                    nc.vector.tensor_copy(
                        dst, base_v[:, k, axis, b_lo * J:(b_lo + 1) * J, 0])
            c0, c1, w0, w1_ = chain(rowp, xy1[:], 4 * PT1, "p1c")
            for wt_ in (w0, w1_):  # zero pad-point weights
                nc.vector.memset(
                    AP(wt_[:].tensor, wt_[:].offset + J,
                       [list(wt_[:].ap[0]), [PT1, 4], [1, PT1 - J]]), 0.0)
            idx_row, w_row = assemble(
                rowp, PT1, (c0[:], c1[:]), (c0[:], c1[:]),
                (w0[:], w1_[:]), (w0[:], w1_[:]),
                lambda b, axis: axis * 2 * PT1 + b * PT1, "p1a")
            idx1.append(wrap_idx(const, idx_row, 2 * N1, f"p1i{k}"))
            w1pair.append(wrap_w(const, w_row, 2 * N1, f"p1w{k}"))

        # ---- main loop over pairs (software-pipelined) ----
        def phase_a(k):
            """load pair k, pass-1 gather, MLP, pass-2 idx/weight prep."""
            feat_t = featp.tile([128, 4 * FREE], f32, tag="feat")
            fpitch = feat_t[:].ap[0][0]
            for b_lo in range(2):
                dst = AP(feat_t[:].tensor,
                         feat_t[:].offset + b_lo * 64 * fpitch,
                         [[fpitch, 64], [1, 4 * FREE]])
                nc.scalar.dma_start(dst, feats[2 * k + b_lo])

            g1 = gath.tile([128, N1 * 4], f32, tag="g1")
            nc.gpsimd.ap_gather(g1[:], feat_t[:], idx1[k][:], channels=128,
                                num_elems=FREE, d=4, num_idxs=N1)
            seed = combine(g1, repl_w(w1pair[k], N1), PT1, N1)

            spitch = seed[:].ap[0][0]
            wpitch = w1q_t[:].ap[0][0]
            h_ps0 = psum1.tile([128, J], f32, tag="hps0")
            h_ps1 = psum1.tile([128, J], f32, tag="hps1")
            h_pss = [h_ps0, h_ps1]
            for b_lo in range(2):
                hsl = h_pss[b_lo][:]
                for cm in range(4):
                    rhs = AP(seed[:].tensor,
                             seed[:].offset + b_lo * 64 * spitch + cm,
                             [[spitch, 64], [4, J]])
                    lhsT = AP(w1q_t[:].tensor,
                              w1q_t[:].offset + b_lo * 64 * wpitch
                              + cm * 128,
                              [[wpitch, 64], [1, 128]])
                    nc.tensor.matmul(hsl, lhsT, rhs, start=(cm == 0),
                                     stop=(cm == 3))
            h_t = gath.tile([128, 2 * J], f32, tag="h")
            for b_lo in range(2):
                nc.scalar.activation(h_t[:, b_lo * J:(b_lo + 1) * J],
                                     h_pss[b_lo][:], Act.Relu,
                                     bias=b1_t[:, 0:1])

            offrow = rowp.tile([1, 4 * PTN], f32, tag="p2off")
            for b_lo in range(2):
                off_ps = psum1.tile([J, O2], f32, tag=f"offps{b_lo}")
                nc.tensor.matmul(off_ps[:], h_t[:, b_lo * J:(b_lo + 1) * J],
                                 w2_t[:], start=True, stop=True)
                off_t = gath.tile([J, O2], f32, tag=f"off{b_lo}")
                nc.vector.tensor_add(off_t[:], off_ps[:], b2_t[0:J, :])
                for axis in range(2):
                    srcap = AP(off_t[:].tensor, off_t[:].offset + axis,
                               [list(off_t[:].ap[0]), [2, NPTS]])
                    nc.sync.dma_start(
                        offrow[:, axis * 2 * PTN + b_lo * PTN:
                               axis * 2 * PTN + (b_lo + 1) * PTN], srcap)

            xy2 = rowp.tile([1, 4 * PTN], f32, tag="p2xy")
            nc.vector.tensor_add(
                xy2[:], offrow[:],
                base_t[:, k * 4 * PTN:(k + 1) * 4 * PTN])
            c0, c1, w0, w1_ = chain(rowp, xy2[:], 4 * PTN, "p2c")
            idx_row, w_row = assemble(
                rowp, PTN, (c0[:], c1[:]), (c0[:], c1[:]),
                (w0[:], w1_[:]), (w0[:], w1_[:]),
                lambda b, axis: axis * 2 * PTN + b * PTN, "p2a")
            idx2 = wrap_idx(wbp, idx_row, 2 * N2, "p2i")
            w2pair = wrap_w(wbp, w_row, 2 * N2, "p2w")
            return feat_t, idx2, w2pair

        def phase_b(k, state):
            feat_t, idx2, w2pair = state
            g2 = gath.tile([128, N2 * 4], f32, tag="g2")
            nc.gpsimd.ap_gather(g2[:], feat_t[:], idx2[:], channels=128,
                                num_elems=FREE, d=4, num_idxs=N2)
            samp = combine(g2, repl_w(w2pair, N2), PTN, N2)

            gpitch = samp[:].ap[0][0]
            ipitch = ident2_t[:].ap[0][0]
            for b_lo in range(2):
                ot = outp.tile([PTN, C], f32, tag="ot")
                ov = ot[:].rearrange("q (cq cm) -> q cq cm", cm=4)
                for cm in range(4):
                    t_ps = psum1.tile([PTN, 64], f32, tag="tps")
                    lhsT = AP(samp[:].tensor,
                              samp[:].offset + b_lo * 64 * gpitch + cm,
                              [[gpitch, 64], [4, PTN]])
                    rhs = AP(ident2_t[:].tensor,
                             ident2_t[:].offset + b_lo * 64 * ipitch,
                             [[ipitch, 64], [1, 64]])
                    nc.tensor.matmul(t_ps[:], lhsT, rhs, is_transpose=True,
                                     start=True, stop=True)
                    nc.vector.tensor_copy(ov[:, :, cm], t_ps[:])
                nc.sync.dma_start(out_v[2 * k + b_lo], ot[:])

        states = {}
        states[0] = phase_a(0)
        states[1] = phase_a(1)
        phase_b(0, states[0])
        states[2] = phase_a(2)
        phase_b(1, states[1])
        states[3] = phase_a(3)
        phase_b(2, states[2])
        phase_b(3, states[3])

    nc.compile()
    return nc


def _host_prep(features, keypoint_coords, w1, b1, w2, b2):
    f32 = np.float32
    # channel-quad-last: [core, b, cq, (y, x, cm)]
    f = np.asarray(features, f32).reshape(NCORES, BPC, 64, 4, H, W)
    f = np.ascontiguousarray(f.transpose(0, 1, 2, 4, 5, 3))
    feats = f.reshape(NCORES, BPC, 64, 4 * H * W)

    pix = (np.asarray(keypoint_coords, f32) + 1.0) * 0.5 * (W - 1)
    bp = pix.reshape(NCORES, NPAIR, 2, J, 2)            # [core,k,b,pt,ax]
    bp = bp.transpose(0, 1, 4, 2, 3)                     # [core,k,ax,b,pt]
    bp = np.repeat(bp[..., None], NPTS, axis=-1)         # [...,n]
    bp = np.ascontiguousarray(
        bp.reshape(NCORES, 1, NPAIR * 2 * 2 * PTN), f32)

    w1T = np.asarray(w1, f32).T                          # [256, 128]
    w1q_half = np.empty((64, 512), f32)
    for cm in range(4):
        w1q_half[:, cm * 128:(cm + 1) * 128] = w1T[cm::4]
    w1q = np.ascontiguousarray(np.tile(w1q_half, (2, 1)))

    w2T = np.ascontiguousarray(np.asarray(w2, f32).T)
    b1c = np.ascontiguousarray(np.asarray(b1, f32)[:, None])
    b2c = np.ascontiguousarray(np.tile(np.asarray(b2, f32)[None, :],
                                       (128, 1)))
    P = np.arange(128)
    K = np.arange(32)
    replb = ((P[None, :] // 64 == K[:, None] // 16) &
             (P[None, :] % 16 == K[:, None] % 16)).astype(f32)
    onesb = (P[None, :] // 64 == np.arange(2)[:, None]).astype(f32)
    ident2 = np.ascontiguousarray(np.tile(np.eye(64, dtype=f32), (2, 1)))

    in_maps = []
    for i in range(NCORES):
        in_maps.append({
            "features": feats[i],
            "base_pix": bp[i],
            "w1q": w1q,
            "w2T": w2T,
            "b1": b1c,
            "b2": b2c,
            "replb": replb,
            "onesb": onesb,
            "ident2": ident2,
        })
    return in_maps


def kernel(features, keypoint_coords, w1, b1, w2, b2):
    global LAST_RESULTS
    from concourse.bass_utils import run_bass_kernel_spmd

    if "nc" not in _CACHE:
        _CACHE["nc"] = _build()
    nc = _CACHE["nc"]
    in_maps = _host_prep(features, keypoint_coords, w1, b1, w2, b2)
    res = run_bass_kernel_spmd(nc, in_maps, core_ids=list(range(NCORES)))
    LAST_RESULTS = res
    out = np.concatenate([res.results[i]["out"] for i in range(NCORES)],
                         axis=0)
    return out.astype(np.float32)


if __name__ == "__main__":
    nc = _build()
    print("build + compile OK")


# revision 27
# speedup vs baseline: 1.0630x; 1.0630x over previous
"""Trainium2 Bass kernel for nn_AdaptiveSampler.

Per batch element b of BT=64:
  1. seed  = bilinear_sample(features[b], keypoints[b])          # [C, J]
  2. h     = relu(w1 @ seed + b1); off = w2 @ h + b2             # [2N, J] px
  3. samp  = bilinear_sample(features[b], keypoints + off)       # [C, J*N]
  4. out[b] = samp rearranged to [J, N*C]

Data-parallel over BT across 8 NeuronCores (8 batches/core, 4 pairs).
Features are host-transposed to channel-quad-last layout: SBUF pair tile
[128 part = (b_lo, c//4), free = (y, x, c%4)], loaded once (memory roofline
~90us/core).  Bilinear gathers use the gpsimd `ap_gather` extended
instruction with d=4 (one int16 index per (sample, neighbor) fetches 4
channels), with per-16-partition index groups carrying each b_lo's indices.
Second-pass indices/weights are computed on-chip from the MLP output
(floor/clip via the 2^23 trick on DVE), wrapped into the [32, n/16] storage
layout by one SBUF DMA and replicated across partitions with one PE matmul.
The 4-neighbor combine is one DVE multiply (step-0 channel broadcast of the
weight tile in PSUM) + one strided reduce.  Output [J*N, C] comes from PE
transposes.
"""

import os
import sys

import numpy as np

sys.path.insert(0, "/opt/trn_rl_repo")

BT, C, H, W = 64, 256, 64, 64
J, NPTS = 17, 4
NCORES = 8
BPC = BT // NCORES          # 8 batches per core
NPAIR = BPC // 2
O2 = 2 * NPTS
PT1 = 20                    # padded pass-1 points per b (17 -> 20)
N1 = PT1 * 4                # pass-1 gather slots per group = 80 (F=5)
PTN = J * NPTS              # pass-2 samples per b = 68
N2 = PTN * 4                # pass-2 gather slots per group = 272 (F=17)
FREE = 4096                 # d=4 units per partition (y, x)
MAGIC = 8388608.0

_CACHE = {}
LAST_RESULTS = None


def _build():
    import concourse.bass as bass
    import concourse.tile as tile
    from concourse import bacc, mybir
    from concourse.ap import AP

    dt = mybir.dt
    f32 = dt.float32
    i16 = dt.int16
    Alu = mybir.AluOpType
    Act = mybir.ActivationFunctionType

    nc = bacc.Bacc("TRN2", target_bir_lowering=False, debug=False,
                   num_devices=NCORES)

    feats = nc.dram_tensor("features", [BPC, 64, 4 * H * W], f32,
                           kind="ExternalInput").ap()
    basep = nc.dram_tensor("base_pix", [1, NPAIR * 2 * 2 * PTN], f32,
                           kind="ExternalInput").ap()
    w1qd = nc.dram_tensor("w1q", [128, 512], f32, kind="ExternalInput").ap()
    w2Td = nc.dram_tensor("w2T", [128, O2], f32, kind="ExternalInput").ap()
    b1d = nc.dram_tensor("b1", [128, 1], f32, kind="ExternalInput").ap()
    b2d = nc.dram_tensor("b2", [128, O2], f32, kind="ExternalInput").ap()
    replbd = nc.dram_tensor("replb", [32, 128], f32,
                            kind="ExternalInput").ap()
    onesbd = nc.dram_tensor("onesb", [2, 128], f32, kind="ExternalInput").ap()
    ident2d = nc.dram_tensor("ident2", [128, 64], f32,
                             kind="ExternalInput").ap()
    out = nc.dram_tensor("out", [BPC, J, NPTS * C], f32,
                         kind="ExternalOutput").ap()

    out_v = out.rearrange("b j (n c) -> b (j n) c", c=C)

    from contextlib import ExitStack

    with tile.TileContext(nc) as tc, ExitStack() as ctx:
        const = ctx.enter_context(tc.tile_pool(name="const", bufs=1))
        featp = ctx.enter_context(tc.tile_pool(name="featp", bufs=2))
        gath = ctx.enter_context(tc.tile_pool(name="gath", bufs=2))
        rowp = ctx.enter_context(tc.tile_pool(name="rowp", bufs=1))
        wbp = ctx.enter_context(tc.tile_pool(name="wbp", bufs=2))
        outp = ctx.enter_context(tc.tile_pool(name="outp", bufs=2))
        psum = ctx.enter_context(tc.tile_pool(name="psum", bufs=2,
                                              space="PSUM"))
        psum1 = ctx.enter_context(tc.tile_pool(name="psum1", bufs=1,
                                               space="PSUM"))

        # ---- constants ----
        w1q_t = const.tile([128, 512], f32, tag="w1q")
        nc.sync.dma_start(w1q_t[:], w1qd)
        w2_t = const.tile([128, O2], f32, tag="w2")
        nc.sync.dma_start(w2_t[:], w2Td)
        b1_t = const.tile([128, 1], f32, tag="b1")
        nc.sync.dma_start(b1_t[:], b1d)
        b2_t = const.tile([128, O2], f32, tag="b2")
        nc.sync.dma_start(b2_t[:], b2d)
        replb_t = const.tile([32, 128], f32, tag="replb")
        nc.sync.dma_start(replb_t[:], replbd)
        onesb_t = const.tile([2, 128], f32, tag="onesb")
        nc.sync.dma_start(onesb_t[:], onesbd)
        ident2_t = const.tile([128, 64], f32, tag="ident2")
        nc.sync.dma_start(ident2_t[:], ident2d)
        base_t = const.tile([1, NPAIR * 2 * 2 * PTN], f32, tag="base")
        nc.sync.dma_start(base_t[:], basep)

        def chain(pool, x, n, tagpfx):
            """floor/clip/validity-weights on a [1, n] row; returns
            (c0, c1, w0, w1) tiles [1, n]."""
            s1 = pool.tile([1, n], f32, tag=tagpfx + "s1")
            s2 = pool.tile([1, n], f32, tag=tagpfx + "s2")
            s3 = pool.tile([1, n], f32, tag=tagpfx + "s3")
            c0 = pool.tile([1, n], f32, tag=tagpfx + "c0")
            c1 = pool.tile([1, n], f32, tag=tagpfx + "c1")
            w0 = pool.tile([1, n], f32, tag=tagpfx + "w0")
            w1t = pool.tile([1, n], f32, tag=tagpfx + "w1")
            hi = float(W - 1)
            nc.vector.tensor_scalar(s1[:], x, MAGIC, None, Alu.add)
            nc.vector.tensor_scalar(s1[:], s1[:], MAGIC, None, Alu.subtract)
            nc.vector.tensor_tensor(s2[:], x, s1[:], Alu.is_lt)
            nc.vector.tensor_sub(s1[:], s1[:], s2[:])              # floor
            nc.vector.tensor_scalar(s3[:], s1[:], 1.0, None, Alu.add)
            nc.vector.tensor_scalar(c0[:], s1[:], 0.0, hi, Alu.max, Alu.min)
            nc.vector.tensor_scalar(c1[:], s3[:], 0.0, hi, Alu.max, Alu.min)
            nc.vector.tensor_tensor(s2[:], s1[:], c0[:], Alu.is_equal)
            nc.vector.tensor_sub(w0[:], s3[:], x)
            nc.vector.tensor_mul(w0[:], w0[:], s2[:])
            nc.vector.tensor_tensor(s2[:], s3[:], c1[:], Alu.is_equal)
            nc.vector.tensor_sub(w1t[:], x, s1[:])
            nc.vector.tensor_mul(w1t[:], w1t[:], s2[:])
            return c0, c1, w0, w1t

        def assemble(pool, npt, cx, cy, wx, wy, xoff, tagpfx):
            """Build e-order idx row [1, 2*npt*4] (b_lo-major halves) and
            slot-order weight row [1, 2*npt*4].

            Per group slot i = pt*4 + q; storage row p = i%16 holds
            e = F*p + i//16 with F = npt/4.  cx/cy/wx/wy are (lo, hi)
            [1, *] rows; samples for (b_lo, axis) start at col
            xoff(b_lo, axis).
            """
            F = npt // 4
            ni = 2 * npt * 4
            idxe = pool.tile([1, ni], f32, tag=tagpfx + "idx")
            wrow = pool.tile([1, ni], f32, tag=tagpfx + "wrow")
            it, io = idxe[:].tensor, idxe[:].offset
            wt, wo = wrow[:].tensor, wrow[:].offset
            for b_lo in range(2):
                for q in range(4):
                    qy, qx = q // 2, q % 2
                    # e-grid: [pm = pt%4 (4), pd = pt//4 (F)]
                    ysrc = AP(cy[qy].tensor, cy[qy].offset + xoff(b_lo, 1),
                              [list(cy[qy].ap[0]), [1, 4], [4, F]])
                    xsrc = AP(cx[qx].tensor, cx[qx].offset + xoff(b_lo, 0),
                              [list(cx[qx].ap[0]), [1, 4], [4, F]])
                    idst = AP(it, io + b_lo * npt * 4 + F * q,
                              [[ni, 1], [4 * F, 4], [1, F]])
                    nc.vector.scalar_tensor_tensor(idst, ysrc, float(W),
                                                   xsrc, Alu.mult, Alu.add)
                    wysrc = AP(wy[qy].tensor, wy[qy].offset + xoff(b_lo, 1),
                               [list(wy[qy].ap[0]), [1, npt]])
                    wxsrc = AP(wx[qx].tensor, wx[qx].offset + xoff(b_lo, 0),
                               [list(wx[qx].ap[0]), [1, npt]])
                    wdst = AP(wt, wo + b_lo * npt * 4 + q,
                              [[ni, 1], [4, npt]])
                    nc.vector.tensor_mul(wdst, wysrc, wxsrc)
            return idxe, wrow

        def wrap_idx(pool, idx_row, ni, tagpfx):
            """e-order f32 idx row [1, ni] -> int16 idx tile [128, ni/32]."""
            nf = ni // 32
            wrapt = pool.tile([32, nf], f32, tag=tagpfx + "wrap")
            nc.scalar.dma_start(wrapt[:], idx_row[:])
            rep_ps = psum1.tile([128, nf], f32, tag="replps")
            nc.tensor.matmul(rep_ps[:], replb_t[:], wrapt[:], start=True,
                             stop=True)
            idxt = pool.tile([128, nf], i16, tag=tagpfx + "idxi")
            nc.vector.tensor_copy(idxt[:], rep_ps[:])
            return idxt

        def wrap_w(pool, w_row, ni, tagpfx):
            """slot-order w row [1, ni] -> [2, ni/2] tile (b_lo rows)."""
            wpair = pool.tile([2, ni // 2], f32, tag=tagpfx + "wpair")
            nc.scalar.dma_start(wpair[:], w_row[:])
            return wpair

        def repl_w(wpair, nf):
            wps = psum1.tile([128, nf], f32, tag="wps")
            nc.tensor.matmul(wps[:], onesb_t[:], wpair[:], start=True,
                             stop=True)
            return wps

        def combine(g, wps, npt, nslots):
            """g [128, nslots*4] (slot, cm) *= w[slot]; reduce over q ->
            [128, npt*4] cols (pt, cm)."""
            gv = g[:].rearrange("p (s c) -> p s c", c=4)
            wb = AP(wps[:].tensor, wps[:].offset,
                    [list(wps[:].ap[0]), [1, nslots], [0, 4]])
            nc.vector.tensor_mul(gv, gv, wb)
            red = gath.tile([128, npt * 4], f32, tag=f"red{nslots}")
            rin = AP(g[:].tensor, g[:].offset,
                     [list(g[:].ap[0]), [16, npt], [1, 4], [4, 4]])
            nc.vector.reduce_sum(red[:].rearrange("p (s c) -> p s c", c=4),
                                 rin, axis=mybir.AxisListType.X)
            return red

        # ---- pass-1 prep (coords only) ----
        base_v = base_t[:].rearrange("a (k x s n) -> a k x s n", k=NPAIR,
                                     x=2, n=4)
        idx1 = []
        w1pair = []
        for k in range(NPAIR):
            xy1 = rowp.tile([1, 4 * PT1], f32, tag="p1xy")
            nc.vector.memset(xy1[:], 0.0)
            for axis in range(2):
                for b_lo in range(2):
                    dst = xy1[:, axis * 2 * PT1 + b_lo * PT1:
                              axis * 2 * PT1 + b_lo * PT1 + J]
                    nc.vector.tensor_copy(
                        dst, base_v[:, k, axis, b_lo * J:(b_lo + 1) * J, 0])
            c0, c1, w0, w1_ = chain(rowp, xy1[:], 4 * PT1, "p1c")
            for wt_ in (w0, w1_):  # zero pad-point weights
                nc.vector.memset(
                    AP(wt_[:].tensor, wt_[:].offset + J,
                       [list(wt_[:].ap[0]), [PT1, 4], [1, PT1 - J]]), 0.0)
            idx_row, w_row = assemble(
                rowp, PT1, (c0[:], c1[:]), (c0[:], c1[:]),
                (w0[:], w1_[:]), (w0[:], w1_[:]),
                lambda b, axis: axis * 2 * PT1 + b * PT1, "p1a")
            idx1.append(wrap_idx(const, idx_row, 2 * N1, f"p1i{k}"))
            w1pair.append(wrap_w(const, w_row, 2 * N1, f"p1w{k}"))

        # ---- main loop over pairs (software-pipelined) ----
        def phase_a(k):
            """load pair k, pass-1 gather, MLP, pass-2 idx/weight prep."""
            feat_t = featp.tile([128, 4 * FREE], f32, tag="feat")
            fpitch = feat_t[:].ap[0][0]
            for b_lo in range(2):
                dst = AP(feat_t[:].tensor,
                         feat_t[:].offset + b_lo * 64 * fpitch,
                         [[fpitch, 64], [1, 4 * FREE]])
                nc.sync.dma_start(dst, feats[2 * k + b_lo])

            g1 = gath.tile([128, N1 * 4], f32, tag="g1")
            nc.gpsimd.ap_gather(g1[:], feat_t[:], idx1[k][:], channels=128,
                                num_elems=FREE, d=4, num_idxs=N1)
            seed = combine(g1, repl_w(w1pair[k], N1), PT1, N1)

            spitch = seed[:].ap[0][0]
            wpitch = w1q_t[:].ap[0][0]
            h_ps0 = psum1.tile([128, J], f32, tag="hps0")
            h_ps1 = psum1.tile([128, J], f32, tag="hps1")
            h_pss = [h_ps0, h_ps1]
            for b_lo in range(2):
                hsl = h_pss[b_lo][:]
                for cm in range(4):
                    rhs = AP(seed[:].tensor,
                             seed[:].offset + b_lo * 64 * spitch + cm,
                             [[spitch, 64], [4, J]])
                    lhsT = AP(w1q_t[:].tensor,
                              w1q_t[:].offset + b_lo * 64 * wpitch
                              + cm * 128,
                              [[wpitch, 64], [1, 128]])
                    nc.tensor.matmul(hsl, lhsT, rhs, start=(cm == 0),
                                     stop=(cm == 3))
            h_t = gath.tile([128, 2 * J], f32, tag="h")
            for b_lo in range(2):
                nc.scalar.activation(h_t[:, b_lo * J:(b_lo + 1) * J],
                                     h_pss[b_lo][:], Act.Relu,
                                     bias=b1_t[:, 0:1])

            offrow = rowp.tile([1, 4 * PTN], f32, tag="p2off")
            for b_lo in range(2):
                off_ps = psum1.tile([J, O2], f32, tag=f"offps{b_lo}")
                nc.tensor.matmul(off_ps[:], h_t[:, b_lo * J:(b_lo + 1) * J],
                                 w2_t[:], start=True, stop=True)
                off_t = gath.tile([J, O2], f32, tag=f"off{b_lo}")
                nc.vector.tensor_add(off_t[:], off_ps[:], b2_t[0:J, :])
                for axis in range(2):
                    srcap = AP(off_t[:].tensor, off_t[:].offset + axis,
                               [list(off_t[:].ap[0]), [2, NPTS]])
                    nc.scalar.dma_start(
                        offrow[:, axis * 2 * PTN + b_lo * PTN:
                               axis * 2 * PTN + (b_lo + 1) * PTN], srcap)

            xy2 = rowp.tile([1, 4 * PTN], f32, tag="p2xy")
            nc.vector.tensor_add(
                xy2[:], offrow[:],
                base_t[:, k * 4 * PTN:(k + 1) * 4 * PTN])
            c0, c1, w0, w1_ = chain(rowp, xy2[:], 4 * PTN, "p2c")
            idx_row, w_row = assemble(
                rowp, PTN, (c0[:], c1[:]), (c0[:], c1[:]),
                (w0[:], w1_[:]), (w0[:], w1_[:]),
                lambda b, axis: axis * 2 * PTN + b * PTN, "p2a")
            idx2 = wrap_idx(wbp, idx_row, 2 * N2, "p2i")
            w2pair = wrap_w(wbp, w_row, 2 * N2, "p2w")
            return feat_t, idx2, w2pair

        def phase_b(k, state):
            feat_t, idx2, w2pair = state
            g2 = gath.tile([128, N2 * 4], f32, tag="g2")
            nc.gpsimd.ap_gather(g2[:], feat_t[:], idx2[:], channels=128,
                                num_elems=FREE, d=4, num_idxs=N2)
            samp = combine(g2, repl_w(w2pair, N2), PTN, N2)

            gpitch = samp[:].ap[0][0]
            ipitch = ident2_t[:].ap[0][0]
            for b_lo in range(2):
                ot = outp.tile([PTN, C], f32, tag="ot")
                ov = ot[:].rearrange("q (cq cm) -> q cq cm", cm=4)
                for cm in range(4):
                    t_ps = psum1.tile([PTN, 64], f32, tag="tps")
                    lhsT = AP(samp[:].tensor,
                              samp[:].offset + b_lo * 64 * gpitch + cm,
                              [[gpitch, 64], [4, PTN]])
                    rhs = AP(ident2_t[:].tensor,
                             ident2_t[:].offset + b_lo * 64 * ipitch,
                             [[ipitch, 64], [1, 64]])
                    nc.tensor.matmul(t_ps[:], lhsT, rhs, is_transpose=True,
                                     start=True, stop=True)
                    nc.vector.tensor_copy(ov[:, :, cm], t_ps[:])
                nc.sync.dma_start(out_v[2 * k + b_lo], ot[:])

        states = {}
        states[0] = phase_a(0)
        states[1] = phase_a(1)
        phase_b(0, states[0])
        states[2] = phase_a(2)
        phase_b(1, states[1])
        states[3] = phase_a(3)
        phase_b(2, states[2])
        phase_b(3, states[3])

    nc.compile()
    return nc


def _host_prep(features, keypoint_coords, w1, b1, w2, b2):
    f32 = np.float32
    # channel-quad-last: [core, b, cq, (y, x, cm)]
    f = np.asarray(features, f32).reshape(NCORES, BPC, 64, 4, H, W)
    f = np.ascontiguousarray(f.transpose(0, 1, 2, 4, 5, 3))
    feats = f.reshape(NCORES, BPC, 64, 4 * H * W)

    pix = (np.asarray(keypoint_coords, f32) + 1.0) * 0.5 * (W - 1)
    bp = pix.reshape(NCORES, NPAIR, 2, J, 2)            # [core,k,b,pt,ax]
    bp = bp.transpose(0, 1, 4, 2, 3)                     # [core,k,ax,b,pt]
    bp = np.repeat(bp[..., None], NPTS, axis=-1)         # [...,n]
    bp = np.ascontiguousarray(
        bp.reshape(NCORES, 1, NPAIR * 2 * 2 * PTN), f32)

    w1T = np.asarray(w1, f32).T                          # [256, 128]
    w1q_half = np.empty((64, 512), f32)
    for cm in range(4):
        w1q_half[:, cm * 128:(cm + 1) * 128] = w1T[cm::4]
    w1q = np.ascontiguousarray(np.tile(w1q_half, (2, 1)))

    w2T = np.ascontiguousarray(np.asarray(w2, f32).T)
    b1c = np.ascontiguousarray(np.asarray(b1, f32)[:, None])
    b2c = np.ascontiguousarray(np.tile(np.asarray(b2, f32)[None, :],
                                       (128, 1)))
    P = np.arange(128)
    K = np.arange(32)
    replb = ((P[None, :] // 64 == K[:, None] // 16) &
             (P[None, :] % 16 == K[:, None] % 16)).astype(f32)
    onesb = (P[None, :] // 64 == np.arange(2)[:, None]).astype(f32)
    ident2 = np.ascontiguousarray(np.tile(np.eye(64, dtype=f32), (2, 1)))

    in_maps = []
    for i in range(NCORES):
        in_maps.append({
            "features": feats[i],
            "base_pix": bp[i],
            "w1q": w1q,
            "w2T": w2T,
            "b1": b1c,
            "b2": b2c,
            "replb": replb,
            "onesb": onesb,
            "ident2": ident2,
        })
    return in_maps


def kernel(features, keypoint_coords, w1, b1, w2, b2):
    global LAST_RESULTS
    from concourse.bass_utils import run_bass_kernel_spmd

    if "nc" not in _CACHE:
        _CACHE["nc"] = _build()
    nc = _CACHE["nc"]
    in_maps = _host_prep(features, keypoint_coords, w1, b1, w2, b2)
    res = run_bass_kernel_spmd(nc, in_maps, core_ids=list(range(NCORES)))
    LAST_RESULTS = res
    out = np.concatenate([res.results[i]["out"] for i in range(NCORES)],
                         axis=0)
    return out.astype(np.float32)


if __name__ == "__main__":
    nc = _build()
    print("build + compile OK")


# revision 31
# speedup vs baseline: 1.2216x; 1.1492x over previous
"""Trainium2 Bass kernel for nn_AdaptiveSampler.

Per batch element b of BT=64:
  1. seed  = bilinear_sample(features[b], keypoints[b])          # [C, J]
  2. h     = relu(w1 @ seed + b1); off = w2 @ h + b2             # [2N, J] px
  3. samp  = bilinear_sample(features[b], keypoints + off)       # [C, J*N]
  4. out[b] = samp rearranged to [J, N*C]

Data-parallel over BT across 8 NeuronCores (8 batches/core, 4 pairs).
Features are host-transposed to channel-quad-last layout: SBUF pair tile
[128 part = (b_lo, c//4), free = (y, x, c%4)], loaded once (memory roofline
~90us/core).  Bilinear gathers use the gpsimd `ap_gather` extended
instruction with d=4 (one int16 index per (sample, neighbor) fetches 4
channels), with per-16-partition index groups carrying each b_lo's indices.
Second-pass indices/weights are computed on-chip from the MLP output
(floor/clip via the 2^23 trick on DVE), wrapped into the [32, n/16] storage
layout by one SBUF DMA and replicated across partitions with one PE matmul.
The 4-neighbor combine is one DVE multiply (step-0 channel broadcast of the
weight tile in PSUM) + one strided reduce.  Output [J*N, C] comes from PE
transposes.
"""

import os
import sys

import numpy as np

sys.path.insert(0, "/opt/trn_rl_repo")

BT, C, H, W = 64, 256, 64, 64
J, NPTS = 17, 4
NCORES = 8
BPC = BT // NCORES          # 8 batches per core
NPAIR = BPC // 2
O2 = 2 * NPTS
PT1 = 20                    # padded pass-1 points per b (17 -> 20)
N1 = PT1 * 4                # pass-1 gather slots per group = 80 (F=5)
PTN = J * NPTS              # pass-2 samples per b = 68
N2 = PTN * 4                # pass-2 gather slots per group = 272 (F=17)
FREE = 4096                 # d=4 units per partition (y, x)
MAGIC = 8388608.0

_CACHE = {}
LAST_RESULTS = None


def _build():
    import concourse.bass as bass
    import concourse.tile as tile
    from concourse import bacc, mybir
    from concourse.ap import AP

    dt = mybir.dt
    f32 = dt.float32
    i16 = dt.int16
    Alu = mybir.AluOpType
    Act = mybir.ActivationFunctionType

    nc = bacc.Bacc("TRN2", target_bir_lowering=False, debug=False,
                   num_devices=NCORES)

    feats = nc.dram_tensor("features", [BPC, 64, 4 * H * W], f32,
                           kind="ExternalInput").ap()
    basep = nc.dram_tensor("base_pix", [1, NPAIR * 2 * 2 * PTN], f32,
                           kind="ExternalInput").ap()
    w1qd = nc.dram_tensor("w1q", [128, 512], f32, kind="ExternalInput").ap()
    w2Td = nc.dram_tensor("w2T", [128, O2], f32, kind="ExternalInput").ap()
    b1d = nc.dram_tensor("b1", [128, 1], f32, kind="ExternalInput").ap()
    b2d = nc.dram_tensor("b2", [128, O2], f32, kind="ExternalInput").ap()
    replbd = nc.dram_tensor("replb", [32, 128], f32,
                            kind="ExternalInput").ap()
    onesbd = nc.dram_tensor("onesb", [2, 128], f32, kind="ExternalInput").ap()
    ident2d = nc.dram_tensor("ident2", [128, 64], f32,
                             kind="ExternalInput").ap()
    out = nc.dram_tensor("out", [BPC, J, NPTS * C], f32,
                         kind="ExternalOutput").ap()

    out_v = out.rearrange("b j (n c) -> b (j n) c", c=C)

    from contextlib import ExitStack

    with tile.TileContext(nc) as tc, ExitStack() as ctx:
        const = ctx.enter_context(tc.tile_pool(name="const", bufs=1))
        featp = ctx.enter_context(tc.tile_pool(name="featp", bufs=2))
        gath = ctx.enter_context(tc.tile_pool(name="gath", bufs=2))
        rowp = ctx.enter_context(tc.tile_pool(name="rowp", bufs=2))
        wbp = ctx.enter_context(tc.tile_pool(name="wbp", bufs=2))
        outp = ctx.enter_context(tc.tile_pool(name="outp", bufs=2))
        psum = ctx.enter_context(tc.tile_pool(name="psum", bufs=2,
                                              space="PSUM"))
        psum1 = ctx.enter_context(tc.tile_pool(name="psum1", bufs=1,
                                               space="PSUM"))

        # ---- constants ----
        w1q_t = const.tile([128, 512], f32, tag="w1q")
        nc.sync.dma_start(w1q_t[:], w1qd)
        w2_t = const.tile([128, O2], f32, tag="w2")
        nc.sync.dma_start(w2_t[:], w2Td)
        b1_t = const.tile([128, 1], f32, tag="b1")
        nc.sync.dma_start(b1_t[:], b1d)
        b2_t = const.tile([128, O2], f32, tag="b2")
        nc.sync.dma_start(b2_t[:], b2d)
        replb_t = const.tile([32, 128], f32, tag="replb")
        nc.sync.dma_start(replb_t[:], replbd)
        onesb_t = const.tile([2, 128], f32, tag="onesb")
        nc.sync.dma_start(onesb_t[:], onesbd)
        ident2_t = const.tile([128, 64], f32, tag="ident2")
        nc.sync.dma_start(ident2_t[:], ident2d)
        base_t = const.tile([1, NPAIR * 2 * 2 * PTN], f32, tag="base")
        nc.sync.dma_start(base_t[:], basep)

        def chain(pool, x, n, tagpfx):
            """floor/clip/validity-weights on a [1, n] row; returns
            (c0, c1, w0, w1) tiles [1, n]."""
            s1 = pool.tile([1, n], f32, tag=tagpfx + "s1")
            s2 = pool.tile([1, n], f32, tag=tagpfx + "s2")
            s3 = pool.tile([1, n], f32, tag=tagpfx + "s3")
            c0 = pool.tile([1, n], f32, tag=tagpfx + "c0")
            c1 = pool.tile([1, n], f32, tag=tagpfx + "c1")
            w0 = pool.tile([1, n], f32, tag=tagpfx + "w0")
            w1t = pool.tile([1, n], f32, tag=tagpfx + "w1")
            hi = float(W - 1)
            nc.vector.tensor_scalar(s1[:], x, MAGIC, None, Alu.add)
            nc.vector.tensor_scalar(s1[:], s1[:], MAGIC, None, Alu.subtract)
            nc.vector.tensor_tensor(s2[:], x, s1[:], Alu.is_lt)
            nc.vector.tensor_sub(s1[:], s1[:], s2[:])              # floor
            nc.vector.tensor_scalar(s3[:], s1[:], 1.0, None, Alu.add)
            nc.vector.tensor_scalar(c0[:], s1[:], 0.0, hi, Alu.max, Alu.min)
            nc.vector.tensor_scalar(c1[:], s3[:], 0.0, hi, Alu.max, Alu.min)
            nc.vector.tensor_tensor(s2[:], s1[:], c0[:], Alu.is_equal)
            nc.vector.tensor_sub(w0[:], s3[:], x)
            nc.vector.tensor_mul(w0[:], w0[:], s2[:])
            nc.vector.tensor_tensor(s2[:], s3[:], c1[:], Alu.is_equal)
            nc.vector.tensor_sub(w1t[:], x, s1[:])
            nc.vector.tensor_mul(w1t[:], w1t[:], s2[:])
            return c0, c1, w0, w1t

        def assemble(pool, npt, cx, cy, wx, wy, xoff, tagpfx):
            """Build e-order idx row [1, 2*npt*4] (b_lo-major halves) and
            slot-order weight row [1, 2*npt*4].

            Per group slot i = pt*4 + q; storage row p = i%16 holds
            e = F*p + i//16 with F = npt/4.  cx/cy/wx/wy are (lo, hi)
            [1, *] rows; samples for (b_lo, axis) start at col
            xoff(b_lo, axis).
            """
            F = npt // 4
            ni = 2 * npt * 4
            idxe = pool.tile([1, ni], f32, tag=tagpfx + "idx")
            wrow = pool.tile([1, ni], f32, tag=tagpfx + "wrow")
            it, io = idxe[:].tensor, idxe[:].offset
            wt, wo = wrow[:].tensor, wrow[:].offset
            for b_lo in range(2):
                for q in range(4):
                    qy, qx = q // 2, q % 2
                    # e-grid: [pm = pt%4 (4), pd = pt//4 (F)]
                    ysrc = AP(cy[qy].tensor, cy[qy].offset + xoff(b_lo, 1),
                              [list(cy[qy].ap[0]), [1, 4], [4, F]])
                    xsrc = AP(cx[qx].tensor, cx[qx].offset + xoff(b_lo, 0),
                              [list(cx[qx].ap[0]), [1, 4], [4, F]])
                    idst = AP(it, io + b_lo * npt * 4 + F * q,
                              [[ni, 1], [4 * F, 4], [1, F]])
                    nc.vector.scalar_tensor_tensor(idst, ysrc, float(W),
                                                   xsrc, Alu.mult, Alu.add)
                    wysrc = AP(wy[qy].tensor, wy[qy].offset + xoff(b_lo, 1),
                               [list(wy[qy].ap[0]), [1, npt]])
                    wxsrc = AP(wx[qx].tensor, wx[qx].offset + xoff(b_lo, 0),
                               [list(wx[qx].ap[0]), [1, npt]])
                    wdst = AP(wt, wo + b_lo * npt * 4 + q,
                              [[ni, 1], [4, npt]])
                    nc.vector.tensor_mul(wdst, wysrc, wxsrc)
            return idxe, wrow

        def wrap_idx(pool, idx_row, ni, tagpfx):
            """e-order f32 idx row [1, ni] -> int16 idx tile [128, ni/32]."""
            nf = ni // 32
            wrapt = pool.tile([32, nf], f32, tag=tagpfx + "wrap")
            nc.sync.dma_start(wrapt[:], idx_row[:])
            rep_ps = psum1.tile([128, nf], f32, tag="replps")
            nc.tensor.matmul(rep_ps[:], replb_t[:], wrapt[:], start=True,
                             stop=True)
            idxt = pool.tile([128, nf], i16, tag=tagpfx + "idxi")
            nc.vector.tensor_copy(idxt[:], rep_ps[:])
            return idxt

        def wrap_w(pool, w_row, ni, tagpfx):
            """slot-order w row [1, ni] -> [2, ni/2] tile (b_lo rows)."""
            wpair = pool.tile([2, ni // 2], f32, tag=tagpfx + "wpair")
            nc.sync.dma_start(wpair[:], w_row[:])
            return wpair

        def repl_w(wpair, nf):
            wps = psum1.tile([128, nf], f32, tag="wps")
            nc.tensor.matmul(wps[:], onesb_t[:], wpair[:], start=True,
                             stop=True)
            return wps

        def combine(g, wps, npt, nslots):
            """g [128, nslots*4] (slot, cm) *= w[slot]; reduce over q ->
            [128, npt*4] cols (pt, cm)."""
            gv = g[:].rearrange("p (s c) -> p s c", c=4)
            wb = AP(wps[:].tensor, wps[:].offset,
                    [list(wps[:].ap[0]), [1, nslots], [0, 4]])
            nc.vector.tensor_mul(gv, gv, wb)
            red = gath.tile([128, npt * 4], f32, tag=f"red{nslots}")
            rin = AP(g[:].tensor, g[:].offset,
                     [list(g[:].ap[0]), [16, npt], [1, 4], [4, 4]])
            nc.vector.reduce_sum(red[:].rearrange("p (s c) -> p s c", c=4),
                                 rin, axis=mybir.AxisListType.X)
            return red

        # ---- pass-1 prep (coords only) ----
        base_v = base_t[:].rearrange("a (k x s n) -> a k x s n", k=NPAIR,
                                     x=2, n=4)
        idx1 = []
        w1pair = []
        for k in range(NPAIR):
            xy1 = rowp.tile([1, 4 * PT1], f32, tag="p1xy")
            nc.vector.memset(xy1[:], 0.0)
            for axis in range(2):
                for b_lo in range(2):
                    dst = xy1[:, axis * 2 * PT1 + b_lo * PT1:
                              axis * 2 * PT1 + b_lo * PT1 + J]
                    nc.vector.tensor_copy(
                        dst, base_v[:, k, axis, b_lo * J:(b_lo + 1) * J, 0])
            c0, c1, w0, w1_ = chain(rowp, xy1[:], 4 * PT1, "p1c")
            for wt_ in (w0, w1_):  # zero pad-point weights
                nc.vector.memset(
                    AP(wt_[:].tensor, wt_[:].offset + J,
                       [list(wt_[:].ap[0]), [PT1, 4], [1, PT1 - J]]), 0.0)
            idx_row, w_row = assemble(
                rowp, PT1, (c0[:], c1[:]), (c0[:], c1[:]),
                (w0[:], w1_[:]), (w0[:], w1_[:]),
                lambda b, axis: axis * 2 * PT1 + b * PT1, "p1a")
            idx1.append(wrap_idx(const, idx_row, 2 * N1, f"p1i{k}"))
            w1pair.append(wrap_w(const, w_row, 2 * N1, f"p1w{k}"))

        # ---- main loop over pairs (software-pipelined) ----
        def phase_a(k):
            """load pair k, pass-1 gather, MLP, pass-2 idx/weight prep."""
            feat_t = featp.tile([128, 4 * FREE], f32, tag="feat")
            fpitch = feat_t[:].ap[0][0]
            for b_lo in range(2):
                dst = AP(feat_t[:].tensor,
                         feat_t[:].offset + b_lo * 64 * fpitch,
                         [[fpitch, 64], [1, 4 * FREE]])
                nc.sync.dma_start(dst, feats[2 * k + b_lo])

            g1 = gath.tile([128, N1 * 4], f32, tag="g1")
            nc.gpsimd.ap_gather(g1[:], feat_t[:], idx1[k][:], channels=128,
                                num_elems=FREE, d=4, num_idxs=N1)
            seed = combine(g1, repl_w(w1pair[k], N1), PT1, N1)

            spitch = seed[:].ap[0][0]
            wpitch = w1q_t[:].ap[0][0]
            h_ps0 = psum1.tile([128, J], f32, tag="hps0")
            h_ps1 = psum1.tile([128, J], f32, tag="hps1")
            h_pss = [h_ps0, h_ps1]
            for b_lo in range(2):
                hsl = h_pss[b_lo][:]
                for cm in range(4):
                    rhs = AP(seed[:].tensor,
                             seed[:].offset + b_lo * 64 * spitch + cm,
                             [[spitch, 64], [4, J]])
                    lhsT = AP(w1q_t[:].tensor,
                              w1q_t[:].offset + b_lo * 64 * wpitch
                              + cm * 128,
                              [[wpitch, 64], [1, 128]])
                    nc.tensor.matmul(hsl, lhsT, rhs, start=(cm == 0),
                                     stop=(cm == 3))
            h_t = gath.tile([128, 2 * J], f32, tag="h")
            for b_lo in range(2):
                nc.scalar.activation(h_t[:, b_lo * J:(b_lo + 1) * J],
                                     h_pss[b_lo][:], Act.Relu,
                                     bias=b1_t[:, 0:1])

            offrow = rowp.tile([1, 4 * PTN], f32, tag="p2off")
            for b_lo in range(2):
                off_ps = psum1.tile([J, O2], f32, tag=f"offps{b_lo}")
                nc.tensor.matmul(off_ps[:], h_t[:, b_lo * J:(b_lo + 1) * J],
                                 w2_t[:], start=True, stop=True)
                off_t = gath.tile([J, O2], f32, tag=f"off{b_lo}")
                nc.vector.tensor_add(off_t[:], off_ps[:], b2_t[0:J, :])
                for axis in range(2):
                    srcap = AP(off_t[:].tensor, off_t[:].offset + axis,
                               [list(off_t[:].ap[0]), [2, NPTS]])
                    nc.sync.dma_start(
                        offrow[:, axis * 2 * PTN + b_lo * PTN:
                               axis * 2 * PTN + (b_lo + 1) * PTN], srcap)

            xy2 = rowp.tile([1, 4 * PTN], f32, tag="p2xy")
            nc.vector.tensor_add(
                xy2[:], offrow[:],
                base_t[:, k * 4 * PTN:(k + 1) * 4 * PTN])
            c0, c1, w0, w1_ = chain(rowp, xy2[:], 4 * PTN, "p2c")
            idx_row, w_row = assemble(
                rowp, PTN, (c0[:], c1[:]), (c0[:], c1[:]),
                (w0[:], w1_[:]), (w0[:], w1_[:]),
                lambda b, axis: axis * 2 * PTN + b * PTN, "p2a")
            idx2 = wrap_idx(wbp, idx_row, 2 * N2, "p2i")
            w2pair = wrap_w(wbp, w_row, 2 * N2, "p2w")
            return feat_t, idx2, w2pair

        def phase_b(k, state):
            feat_t, idx2, w2pair = state
            g2 = gath.tile([128, N2 * 4], f32, tag="g2")
            nc.gpsimd.ap_gather(g2[:], feat_t[:], idx2[:], channels=128,
                                num_elems=FREE, d=4, num_idxs=N2)
            samp = combine(g2, repl_w(w2pair, N2), PTN, N2)

            gpitch = samp[:].ap[0][0]
            ipitch = ident2_t[:].ap[0][0]
            for b_lo in range(2):
                ot = outp.tile([PTN, C], f32, tag="ot")
                ov = ot[:].rearrange("q (cq cm) -> q cq cm", cm=4)
                for cm in range(4):
                    t_ps = psum1.tile([PTN, 64], f32, tag="tps")
                    lhsT = AP(samp[:].tensor,
                              samp[:].offset + b_lo * 64 * gpitch + cm,
                              [[gpitch, 64], [4, PTN]])
                    rhs = AP(ident2_t[:].tensor,
                             ident2_t[:].offset + b_lo * 64 * ipitch,
                             [[ipitch, 64], [1, 64]])
                    nc.tensor.matmul(t_ps[:], lhsT, rhs, is_transpose=True,
                                     start=True, stop=True)
                    nc.vector.tensor_copy(ov[:, :, cm], t_ps[:])
                nc.sync.dma_start(out_v[2 * k + b_lo], ot[:])

        states = {}
        states[0] = phase_a(0)
        states[1] = phase_a(1)
        phase_b(0, states[0])
        states[2] = phase_a(2)
        phase_b(1, states[1])
        states[3] = phase_a(3)
        phase_b(2, states[2])
        phase_b(3, states[3])

    nc.compile()
    return nc


def _host_prep(features, keypoint_coords, w1, b1, w2, b2):
    f32 = np.float32
    # channel-quad-last: [core, b, cq, (y, x, cm)]
    f = np.asarray(features, f32).reshape(NCORES, BPC, 64, 4, H, W)
    f = np.ascontiguousarray(f.transpose(0, 1, 2, 4, 5, 3))
    feats = f.reshape(NCORES, BPC, 64, 4 * H * W)

    pix = (np.asarray(keypoint_coords, f32) + 1.0) * 0.5 * (W - 1)
    bp = pix.reshape(NCORES, NPAIR, 2, J, 2)            # [core,k,b,pt,ax]
    bp = bp.transpose(0, 1, 4, 2, 3)                     # [core,k,ax,b,pt]
    bp = np.repeat(bp[..., None], NPTS, axis=-1)         # [...,n]
    bp = np.ascontiguousarray(
        bp.reshape(NCORES, 1, NPAIR * 2 * 2 * PTN), f32)

    w1T = np.asarray(w1, f32).T                          # [256, 128]
    w1q_half = np.empty((64, 512), f32)
    for cm in range(4):
        w1q_half[:, cm * 128:(cm + 1) * 128] = w1T[cm::4]
    w1q = np.ascontiguousarray(np.tile(w1q_half, (2, 1)))

    w2T = np.ascontiguousarray(np.asarray(w2, f32).T)
    b1c = np.ascontiguousarray(np.asarray(b1, f32)[:, None])
    b2c = np.ascontiguousarray(np.tile(np.asarray(b2, f32)[None, :],
                                       (128, 1)))
    P = np.arange(128)
    K = np.arange(32)
    replb = ((P[None, :] // 64 == K[:, None] // 16) &
             (P[None, :] % 16 == K[:, None] % 16)).astype(f32)
    onesb = (P[None, :] // 64 == np.arange(2)[:, None]).astype(f32)
    ident2 = np.ascontiguousarray(np.tile(np.eye(64, dtype=f32), (2, 1)))

    in_maps = []
    for i in range(NCORES):
        in_maps.append({
            "features": feats[i],
            "base_pix": bp[i],
            "w1q": w1q,
            "w2T": w2T,
            "b1": b1c,
            "b2": b2c,
            "replb": replb,
            "onesb": onesb,
            "ident2": ident2,
        })
    return in_maps


def kernel(features, keypoint_coords, w1, b1, w2, b2):
    global LAST_RESULTS
    from concourse.bass_utils import run_bass_kernel_spmd

    if "nc" not in _CACHE:
        _CACHE["nc"] = _build()
    nc = _CACHE["nc"]
    in_maps = _host_prep(features, keypoint_coords, w1, b1, w2, b2)
    res = run_bass_kernel_spmd(nc, in_maps, core_ids=list(range(NCORES)))
    LAST_RESULTS = res
    out = np.concatenate([res.results[i]["out"] for i in range(NCORES)],
                         axis=0)
    return out.astype(np.float32)


if __name__ == "__main__":
    nc = _build()
    print("build + compile OK")


# revision 32
# speedup vs baseline: 1.2216x; 1.0000x over previous
"""Trainium2 Bass kernel for nn_AdaptiveSampler.

Per batch element b of BT=64:
  1. seed  = bilinear_sample(features[b], keypoints[b])          # [C, J]
  2. h     = relu(w1 @ seed + b1); off = w2 @ h + b2             # [2N, J] px
  3. samp  = bilinear_sample(features[b], keypoints + off)       # [C, J*N]
  4. out[b] = samp rearranged to [J, N*C]

Data-parallel over BT across 8 NeuronCores (8 batches/core, 4 pairs).
Features are host-transposed to channel-quad-last layout: SBUF pair tile
[128 part = (b_lo, c//4), free = (y, x, c%4)], loaded once (memory roofline
~90us/core).  Bilinear gathers use the gpsimd `ap_gather` extended
instruction with d=4 (one int16 index per (sample, neighbor) fetches 4
channels), with per-16-partition index groups carrying each b_lo's indices.
Second-pass indices/weights are computed on-chip from the MLP output
(floor/clip via the 2^23 trick on DVE), wrapped into the [32, n/16] storage
layout by one SBUF DMA and replicated across partitions with one PE matmul.
The 4-neighbor combine is one DVE multiply (step-0 channel broadcast of the
weight tile in PSUM) + one strided reduce.  Output [J*N, C] comes from PE
transposes.
"""

import os
import sys

import numpy as np

sys.path.insert(0, "/opt/trn_rl_repo")

BT, C, H, W = 64, 256, 64, 64
J, NPTS = 17, 4
NCORES = 8
BPC = BT // NCORES          # 8 batches per core
NPAIR = BPC // 2
O2 = 2 * NPTS
PT1 = 20                    # padded pass-1 points per b (17 -> 20)
N1 = PT1 * 4                # pass-1 gather slots per group = 80 (F=5)
PTN = J * NPTS              # pass-2 samples per b = 68
N2 = PTN * 4                # pass-2 gather slots per group = 272 (F=17)
FREE = 4096                 # d=4 units per partition (y, x)
MAGIC = 8388608.0

_CACHE = {}
LAST_RESULTS = None


def _build():
    import concourse.bass as bass
    import concourse.tile as tile
    from concourse import bacc, mybir
    from concourse.ap import AP

    dt = mybir.dt
    f32 = dt.float32
    i16 = dt.int16
    Alu = mybir.AluOpType
    Act = mybir.ActivationFunctionType

    nc = bacc.Bacc("TRN2", target_bir_lowering=False, debug=False,
                   num_devices=NCORES)

    feats = nc.dram_tensor("features", [BPC, 64, 4 * H * W], f32,
                           kind="ExternalInput").ap()
    basep = nc.dram_tensor("base_pix", [1, NPAIR * 2 * 2 * PTN], f32,
                           kind="ExternalInput").ap()
    w1qd = nc.dram_tensor("w1q", [128, 512], f32, kind="ExternalInput").ap()
    w2Td = nc.dram_tensor("w2T", [128, O2], f32, kind="ExternalInput").ap()
    b1d = nc.dram_tensor("b1", [128, 1], f32, kind="ExternalInput").ap()
    b2d = nc.dram_tensor("b2", [128, O2], f32, kind="ExternalInput").ap()
    replbd = nc.dram_tensor("replb", [32, 128], f32,
                            kind="ExternalInput").ap()
    onesbd = nc.dram_tensor("onesb", [2, 128], f32, kind="ExternalInput").ap()
    ident2d = nc.dram_tensor("ident2", [128, 64], f32,
                             kind="ExternalInput").ap()
    out = nc.dram_tensor("out", [BPC, J, NPTS * C], f32,
                         kind="ExternalOutput").ap()

    out_v = out.rearrange("b j (n c) -> b (j n) c", c=C)

    from contextlib import ExitStack

    with tile.TileContext(nc) as tc, ExitStack() as ctx:
        const = ctx.enter_context(tc.tile_pool(name="const", bufs=1))
        featp = ctx.enter_context(tc.tile_pool(name="featp", bufs=2))
        gath = ctx.enter_context(tc.tile_pool(name="gath", bufs=2))
        rowp = ctx.enter_context(tc.tile_pool(name="rowp", bufs=2))
        wbp = ctx.enter_context(tc.tile_pool(name="wbp", bufs=2))
        outp = ctx.enter_context(tc.tile_pool(name="outp", bufs=2))
        psum = ctx.enter_context(tc.tile_pool(name="psum", bufs=2,
                                              space="PSUM"))
        psum1 = ctx.enter_context(tc.tile_pool(name="psum1", bufs=1,
                                               space="PSUM"))

        # preload the ap_gather Q7 library before bulk DMAs queue up
        from concourse import library_config
        nc.gpsimd.load_library(library_config.ap_gather)

        # ---- constants ----
        w1q_t = const.tile([128, 512], f32, tag="w1q")
        nc.sync.dma_start(w1q_t[:], w1qd)
        w2_t = const.tile([128, O2], f32, tag="w2")
        nc.sync.dma_start(w2_t[:], w2Td)
        b1_t = const.tile([128, 1], f32, tag="b1")
        nc.sync.dma_start(b1_t[:], b1d)
        b2_t = const.tile([128, O2], f32, tag="b2")
        nc.sync.dma_start(b2_t[:], b2d)
        replb_t = const.tile([32, 128], f32, tag="replb")
        nc.sync.dma_start(replb_t[:], replbd)
        onesb_t = const.tile([2, 128], f32, tag="onesb")
        nc.sync.dma_start(onesb_t[:], onesbd)
        ident2_t = const.tile([128, 64], f32, tag="ident2")
        nc.sync.dma_start(ident2_t[:], ident2d)
        base_t = const.tile([1, NPAIR * 2 * 2 * PTN], f32, tag="base")
        nc.sync.dma_start(base_t[:], basep)

        def chain(pool, x, n, tagpfx):
            """floor/clip/validity-weights on a [1, n] row; returns
            (c0, c1, w0, w1) tiles [1, n]."""
            s1 = pool.tile([1, n], f32, tag=tagpfx + "s1")
            s2 = pool.tile([1, n], f32, tag=tagpfx + "s2")
            s3 = pool.tile([1, n], f32, tag=tagpfx + "s3")
            c0 = pool.tile([1, n], f32, tag=tagpfx + "c0")
            c1 = pool.tile([1, n], f32, tag=tagpfx + "c1")
            w0 = pool.tile([1, n], f32, tag=tagpfx + "w0")
            w1t = pool.tile([1, n], f32, tag=tagpfx + "w1")
            hi = float(W - 1)
            nc.vector.tensor_scalar(s1[:], x, MAGIC, None, Alu.add)
            nc.vector.tensor_scalar(s1[:], s1[:], MAGIC, None, Alu.subtract)
            nc.vector.tensor_tensor(s2[:], x, s1[:], Alu.is_lt)
            nc.vector.tensor_sub(s1[:], s1[:], s2[:])              # floor
            nc.vector.tensor_scalar(s3[:], s1[:], 1.0, None, Alu.add)
            nc.vector.tensor_scalar(c0[:], s1[:], 0.0, hi, Alu.max, Alu.min)
            nc.vector.tensor_scalar(c1[:], s3[:], 0.0, hi, Alu.max, Alu.min)
            nc.vector.tensor_tensor(s2[:], s1[:], c0[:], Alu.is_equal)
            nc.vector.tensor_sub(w0[:], s3[:], x)
            nc.vector.tensor_mul(w0[:], w0[:], s2[:])
            nc.vector.tensor_tensor(s2[:], s3[:], c1[:], Alu.is_equal)
            nc.vector.tensor_sub(w1t[:], x, s1[:])
            nc.vector.tensor_mul(w1t[:], w1t[:], s2[:])
            return c0, c1, w0, w1t

        def assemble(pool, npt, cx, cy, wx, wy, xoff, tagpfx):
            """Build e-order idx row [1, 2*npt*4] (b_lo-major halves) and
            slot-order weight row [1, 2*npt*4].

            Per group slot i = pt*4 + q; storage row p = i%16 holds
            e = F*p + i//16 with F = npt/4.  cx/cy/wx/wy are (lo, hi)
            [1, *] rows; samples for (b_lo, axis) start at col
            xoff(b_lo, axis).
            """
            F = npt // 4
            ni = 2 * npt * 4
            idxe = pool.tile([1, ni], f32, tag=tagpfx + "idx")
            wrow = pool.tile([1, ni], f32, tag=tagpfx + "wrow")
            it, io = idxe[:].tensor, idxe[:].offset
            wt, wo = wrow[:].tensor, wrow[:].offset
            for b_lo in range(2):
                for q in range(4):
                    qy, qx = q // 2, q % 2
                    # e-grid: [pm = pt%4 (4), pd = pt//4 (F)]
                    ysrc = AP(cy[qy].tensor, cy[qy].offset + xoff(b_lo, 1),
                              [list(cy[qy].ap[0]), [1, 4], [4, F]])
                    xsrc = AP(cx[qx].tensor, cx[qx].offset + xoff(b_lo, 0),
                              [list(cx[qx].ap[0]), [1, 4], [4, F]])
                    idst = AP(it, io + b_lo * npt * 4 + F * q,
                              [[ni, 1], [4 * F, 4], [1, F]])
                    nc.vector.scalar_tensor_tensor(idst, ysrc, float(W),
                                                   xsrc, Alu.mult, Alu.add)
                    wysrc = AP(wy[qy].tensor, wy[qy].offset + xoff(b_lo, 1),
                               [list(wy[qy].ap[0]), [1, npt]])
                    wxsrc = AP(wx[qx].tensor, wx[qx].offset + xoff(b_lo, 0),
                               [list(wx[qx].ap[0]), [1, npt]])
                    wdst = AP(wt, wo + b_lo * npt * 4 + q,
                              [[ni, 1], [4, npt]])
                    nc.vector.tensor_mul(wdst, wysrc, wxsrc)
            return idxe, wrow

        def wrap_idx(pool, idx_row, ni, tagpfx):
            """e-order f32 idx row [1, ni] -> int16 idx tile [128, ni/32]."""
            nf = ni // 32
            wrapt = pool.tile([32, nf], f32, tag=tagpfx + "wrap")
            nc.sync.dma_start(wrapt[:], idx_row[:])
            rep_ps = psum1.tile([128, nf], f32, tag="replps")
            nc.tensor.matmul(rep_ps[:], replb_t[:], wrapt[:], start=True,
                             stop=True)
            idxt = pool.tile([128, nf], i16, tag=tagpfx + "idxi")
            nc.vector.tensor_copy(idxt[:], rep_ps[:])
            return idxt

        def wrap_w(pool, w_row, ni, tagpfx):
            """slot-order w row [1, ni] -> [2, ni/2] tile (b_lo rows)."""
            wpair = pool.tile([2, ni // 2], f32, tag=tagpfx + "wpair")
            nc.sync.dma_start(wpair[:], w_row[:])
            return wpair

        def repl_w(wpair, nf):
            wps = psum1.tile([128, nf], f32, tag="wps")
            nc.tensor.matmul(wps[:], onesb_t[:], wpair[:], start=True,
                             stop=True)
            return wps

        def combine(g, wps, npt, nslots):
            """g [128, nslots*4] (slot, cm) *= w[slot]; reduce over q ->
            [128, npt*4] cols (pt, cm)."""
            gv = g[:].rearrange("p (s c) -> p s c", c=4)
            wb = AP(wps[:].tensor, wps[:].offset,
                    [list(wps[:].ap[0]), [1, nslots], [0, 4]])
            nc.vector.tensor_mul(gv, gv, wb)
            red = gath.tile([128, npt * 4], f32, tag=f"red{nslots}")
            rin = AP(g[:].tensor, g[:].offset,
                     [list(g[:].ap[0]), [16, npt], [1, 4], [4, 4]])
            nc.vector.reduce_sum(red[:].rearrange("p (s c) -> p s c", c=4),
                                 rin, axis=mybir.AxisListType.X)
            return red

        # ---- pass-1 prep (coords only) ----
        base_v = base_t[:].rearrange("a (k x s n) -> a k x s n", k=NPAIR,
                                     x=2, n=4)
        idx1 = []
        w1pair = []
        for k in range(NPAIR):
            xy1 = rowp.tile([1, 4 * PT1], f32, tag="p1xy")
            nc.vector.memset(xy1[:], 0.0)
            for axis in range(2):
                for b_lo in range(2):
                    dst = xy1[:, axis * 2 * PT1 + b_lo * PT1:
                              axis * 2 * PT1 + b_lo * PT1 + J]
                    nc.vector.tensor_copy(
                        dst, base_v[:, k, axis, b_lo * J:(b_lo + 1) * J, 0])
            c0, c1, w0, w1_ = chain(rowp, xy1[:], 4 * PT1, "p1c")
            for wt_ in (w0, w1_):  # zero pad-point weights
                nc.vector.memset(
                    AP(wt_[:].tensor, wt_[:].offset + J,
                       [list(wt_[:].ap[0]), [PT1, 4], [1, PT1 - J]]), 0.0)
            idx_row, w_row = assemble(
                rowp, PT1, (c0[:], c1[:]), (c0[:], c1[:]),
                (w0[:], w1_[:]), (w0[:], w1_[:]),
                lambda b, axis: axis * 2 * PT1 + b * PT1, "p1a")
            idx1.append(wrap_idx(const, idx_row, 2 * N1, f"p1i{k}"))
            w1pair.append(wrap_w(const, w_row, 2 * N1, f"p1w{k}"))

        # ---- main loop over pairs (software-pipelined) ----
        def phase_a(k):
            """load pair k, pass-1 gather, MLP, pass-2 idx/weight prep."""
            feat_t = featp.tile([128, 4 * FREE], f32, tag="feat")
            fpitch = feat_t[:].ap[0][0]
            for b_lo in range(2):
                dst = AP(feat_t[:].tensor,
                         feat_t[:].offset + b_lo * 64 * fpitch,
                         [[fpitch, 64], [1, 4 * FREE]])
                nc.sync.dma_start(dst, feats[2 * k + b_lo])

            g1 = gath.tile([128, N1 * 4], f32, tag="g1")
            nc.gpsimd.ap_gather(g1[:], feat_t[:], idx1[k][:], channels=128,
                                num_elems=FREE, d=4, num_idxs=N1)
            seed = combine(g1, repl_w(w1pair[k], N1), PT1, N1)

            spitch = seed[:].ap[0][0]
            wpitch = w1q_t[:].ap[0][0]
            h_ps0 = psum1.tile([128, J], f32, tag="hps0")
            h_ps1 = psum1.tile([128, J], f32, tag="hps1")
            h_pss = [h_ps0, h_ps1]
            for b_lo in range(2):
                hsl = h_pss[b_lo][:]
                for cm in range(4):
                    rhs = AP(seed[:].tensor,
                             seed[:].offset + b_lo * 64 * spitch + cm,
                             [[spitch, 64], [4, J]])
                    lhsT = AP(w1q_t[:].tensor,
                              w1q_t[:].offset + b_lo * 64 * wpitch
                              + cm * 128,
                              [[wpitch, 64], [1, 128]])
                    nc.tensor.matmul(hsl, lhsT, rhs, start=(cm == 0),
                                     stop=(cm == 3))
            h_t = gath.tile([128, 2 * J], f32, tag="h")
            for b_lo in range(2):
                nc.scalar.activation(h_t[:, b_lo * J:(b_lo + 1) * J],
                                     h_pss[b_lo][:], Act.Relu,
                                     bias=b1_t[:, 0:1])

            offrow = rowp.tile([1, 4 * PTN], f32, tag="p2off")
            for b_lo in range(2):
                off_ps = psum1.tile([J, O2], f32, tag=f"offps{b_lo}")
                nc.tensor.matmul(off_ps[:], h_t[:, b_lo * J:(b_lo + 1) * J],
                                 w2_t[:], start=True, stop=True)
                off_t = gath.tile([J, O2], f32, tag=f"off{b_lo}")
                nc.vector.tensor_add(off_t[:], off_ps[:], b2_t[0:J, :])
                for axis in range(2):
                    srcap = AP(off_t[:].tensor, off_t[:].offset + axis,
                               [list(off_t[:].ap[0]), [2, NPTS]])
                    nc.sync.dma_start(
                        offrow[:, axis * 2 * PTN + b_lo * PTN:
                               axis * 2 * PTN + (b_lo + 1) * PTN], srcap)

            xy2 = rowp.tile([1, 4 * PTN], f32, tag="p2xy")
            nc.vector.tensor_add(
                xy2[:], offrow[:],
                base_t[:, k * 4 * PTN:(k + 1) * 4 * PTN])
            c0, c1, w0, w1_ = chain(rowp, xy2[:], 4 * PTN, "p2c")
            idx_row, w_row = assemble(
                rowp, PTN, (c0[:], c1[:]), (c0[:], c1[:]),
                (w0[:], w1_[:]), (w0[:], w1_[:]),
                lambda b, axis: axis * 2 * PTN + b * PTN, "p2a")
            idx2 = wrap_idx(wbp, idx_row, 2 * N2, "p2i")
            w2pair = wrap_w(wbp, w_row, 2 * N2, "p2w")
            return feat_t, idx2, w2pair

        def phase_b(k, state):
            feat_t, idx2, w2pair = state
            g2 = gath.tile([128, N2 * 4], f32, tag="g2")
            nc.gpsimd.ap_gather(g2[:], feat_t[:], idx2[:], channels=128,
                                num_elems=FREE, d=4, num_idxs=N2)
            samp = combine(g2, repl_w(w2pair, N2), PTN, N2)

            gpitch = samp[:].ap[0][0]
            ipitch = ident2_t[:].ap[0][0]
            for b_lo in range(2):
                ot = outp.tile([PTN, C], f32, tag="ot")
                ov = ot[:].rearrange("q (cq cm) -> q cq cm", cm=4)
                for cm in range(4):
                    t_ps = psum1.tile([PTN, 64], f32, tag="tps")
                    lhsT = AP(samp[:].tensor,
                              samp[:].offset + b_lo * 64 * gpitch + cm,
                              [[gpitch, 64], [4, PTN]])
                    rhs = AP(ident2_t[:].tensor,
                             ident2_t[:].offset + b_lo * 64 * ipitch,
                             [[ipitch, 64], [1, 64]])
                    nc.tensor.matmul(t_ps[:], lhsT, rhs, is_transpose=True,
                                     start=True, stop=True)
                    nc.vector.tensor_copy(ov[:, :, cm], t_ps[:])
                nc.sync.dma_start(out_v[2 * k + b_lo], ot[:])

        states = {}
        states[0] = phase_a(0)
        states[1] = phase_a(1)
        phase_b(0, states[0])
        states[2] = phase_a(2)
        phase_b(1, states[1])
        states[3] = phase_a(3)
        phase_b(2, states[2])
        phase_b(3, states[3])

    nc.compile()
    return nc


def _host_prep(features, keypoint_coords, w1, b1, w2, b2):
    f32 = np.float32
    # channel-quad-last: [core, b, cq, (y, x, cm)]
    f = np.asarray(features, f32).reshape(NCORES, BPC, 64, 4, H, W)
    f = np.ascontiguousarray(f.transpose(0, 1, 2, 4, 5, 3))
    feats = f.reshape(NCORES, BPC, 64, 4 * H * W)

    pix = (np.asarray(keypoint_coords, f32) + 1.0) * 0.5 * (W - 1)
    bp = pix.reshape(NCORES, NPAIR, 2, J, 2)            # [core,k,b,pt,ax]
    bp = bp.transpose(0, 1, 4, 2, 3)                     # [core,k,ax,b,pt]
    bp = np.repeat(bp[..., None], NPTS, axis=-1)         # [...,n]
    bp = np.ascontiguousarray(
        bp.reshape(NCORES, 1, NPAIR * 2 * 2 * PTN), f32)

    w1T = np.asarray(w1, f32).T                          # [256, 128]
    w1q_half = np.empty((64, 512), f32)
    for cm in range(4):
        w1q_half[:, cm * 128:(cm + 1) * 128] = w1T[cm::4]
    w1q = np.ascontiguousarray(np.tile(w1q_half, (2, 1)))

    w2T = np.ascontiguousarray(np.asarray(w2, f32).T)
    b1c = np.ascontiguousarray(np.asarray(b1, f32)[:, None])
    b2c = np.ascontiguousarray(np.tile(np.asarray(b2, f32)[None, :],
                                       (128, 1)))
    P = np.arange(128)
    K = np.arange(32)
    replb = ((P[None, :] // 64 == K[:, None] // 16) &
             (P[None, :] % 16 == K[:, None] % 16)).astype(f32)
    onesb = (P[None, :] // 64 == np.arange(2)[:, None]).astype(f32)
    ident2 = np.ascontiguousarray(np.tile(np.eye(64, dtype=f32), (2, 1)))

    in_maps = []
    for i in range(NCORES):
        in_maps.append({
            "features": feats[i],
            "base_pix": bp[i],
            "w1q": w1q,
            "w2T": w2T,
            "b1": b1c,
            "b2": b2c,
            "replb": replb,
            "onesb": onesb,
            "ident2": ident2,
        })
    return in_maps


def kernel(features, keypoint_coords, w1, b1, w2, b2):
    global LAST_RESULTS
    from concourse.bass_utils import run_bass_kernel_spmd

    if "nc" not in _CACHE:
        _CACHE["nc"] = _build()
    nc = _CACHE["nc"]
    in_maps = _host_prep(features, keypoint_coords, w1, b1, w2, b2)
    res = run_bass_kernel_spmd(nc, in_maps, core_ids=list(range(NCORES)))
    LAST_RESULTS = res
    out = np.concatenate([res.results[i]["out"] for i in range(NCORES)],
                         axis=0)
    return out.astype(np.float32)


if __name__ == "__main__":
    nc = _build()
    print("build + compile OK")
